# revision 15
# baseline (speedup 1.0000x reference)
"""Trainium2 Bass kernel for nn_DeformSimpleBottleneck.

Sharding: 8 cores = (batch b in 0..3) x (row-half in 0..1). Each core computes
its batch's conv1 on the full 64x64 grid (needed for unrestricted deformable
sampling), then the offset conv / deformable conv / conv3 for its 30 rows of
the 60x60 inner grid, and 32 output rows (30 deform rows + 2 border rows).

Deformable bilinear sampling: a duplicated-row DRAM layout y2 (row (g,p) =
[y[p], y[p+64]] for group g's channels) lets one indirect-DMA index fetch all
4 bilinear corners as one contiguous 2KB block. Corner weights (bilinear x
mask x validity) are folded into 4 per-sample scalars applied with fused
scalar_tensor_tensor MACs; clamped indices + zero slot weights reproduce the
reference's out-of-bounds zeroing exactly.
"""
import sys
import numpy as np
import ml_dtypes

sys.path.insert(0, "/opt/trn_rl_repo")
sys.path.insert(0, "/opt/trn_rl_repo/concourse")

import concourse.bass as bass
import concourse.bacc as bacc
import concourse.mybir as mybir
import concourse.tile as tile
from concourse.bass_utils import run_bass_kernel_spmd

f32 = mybir.dt.float32
bf16 = mybir.dt.bfloat16
i32 = mybir.dt.int32
AF = mybir.ActivationFunctionType
OP = mybir.AluOpType

NPIX = 4096
F = 512
C1 = 256
NROW = 30
NP = NROW * 60     # 1800
NT = 15
EPS = 1e-5
Y2ROWS = 2 * 4096

_CACHE = {}


def sap(tile_ap, part_off, part_cnt, free_off, free_dims):
    """SBUF AP from a tile's base AP: partition offset/count + free dims."""
    pstep = tile_ap.ap[0][0]
    return bass.AP(tile_ap.tensor, tile_ap.offset + part_off * pstep + free_off,
                   [[pstep, part_cnt]] + free_dims)


def dap(base_ap, off, dims):
    """DRAM AP at element offset with explicit dims."""
    return bass.AP(base_ap.tensor, base_ap.offset + off, dims)


def build_nc():
    nc = bacc.Bacc("TRN2", target_bir_lowering=False, debug=False, num_devices=8)

    def inp(name, shape, dt=f32):
        return nc.declare_dram_parameter(name, shape, dt, isOutput=False)

    xtb_d = inp("xtb", [128, 2 * NPIX], bf16)
    xts_d = inp("xts", [128, 2 * 2176], bf16)
    xres_d = inp("xres", [128, 2 * NROW * 64], f32)
    xbrd_d = inp("xbrd", [128, 2 * 128], f32)
    w1_d = inp("w1b", [128, 2 * F], bf16)
    woff_d = inp("woff", [128, 36 * 54], bf16)
    bia_d = inp("bia", [54, 1], f32)
    wk_d = inp("wk", [128, 36 * F], bf16)
    w3_d = inp("w3b", [128, 4 * C1], bf16)
    bn1_d = inp("bn1", [128, 4 * 4])
    bn2_d = inp("bn2", [128, 4 * 4])
    bn3_d = inp("bn3", [128, 2 * 4])
    pyb_d = inp("pyb", [128, NT * 18])
    pxb_d = inp("pxb", [128, NT * 18])
    gof_d = inp("gof", [128, 18])
    rmk_d = inp("rmk", [54, 1], mybir.dt.uint8)
    idf_d = inp("idf", [128, 128])
    idb_d = inp("idb", [128, 128], bf16)
    outA_d = nc.declare_dram_parameter("outA", [NROW * 64, C1], f32, isOutput=True)
    outB_d = nc.declare_dram_parameter("outB", [2 * 64, C1], f32, isOutput=True)

    y2_d = nc.dram_tensor("y2", [Y2ROWS * 512 + 4096], bf16)
    y2col = y2_d.ap().rearrange("(a b) -> a b", b=1)
    y2flat = y2_d.ap()

    with tile.TileContext(nc) as tc:
        cp = tc.alloc_tile_pool(name="consts", bufs=1)
        w1 = cp.tile([128, 2, F], bf16)
        nc.sync.dma_start(out=w1[:, :, :], in_=w1_d[:])
        woff = cp.tile([128, 36, 54], bf16)
        nc.sync.dma_start(out=woff[:, :, :], in_=woff_d[:])
        bia = cp.tile([54, 1], f32)
        nc.sync.dma_start(out=bia[:], in_=bia_d[:])
        wk = cp.tile([128, 36, F], bf16)
        nc.sync.dma_start(out=wk[:, :, :], in_=wk_d[:])
        w3 = cp.tile([128, 4, C1], bf16)
        nc.sync.dma_start(out=w3[:, :, :], in_=w3_d[:])
        idf = cp.tile([128, 128], f32)
        nc.sync.dma_start(out=idf[:], in_=idf_d[:])
        idb = cp.tile([128, 128], bf16)
        nc.sync.dma_start(out=idb[:], in_=idb_d[:])
        pyb = cp.tile([128, NT, 18], f32)
        nc.sync.dma_start(out=pyb[:, :, :], in_=pyb_d[:])
        pxb = cp.tile([128, NT, 18], f32)
        nc.sync.dma_start(out=pxb[:, :, :], in_=pxb_d[:])
        gof = cp.tile([128, 18], f32)
        nc.sync.dma_start(out=gof[:], in_=gof_d[:])
        zt = cp.tile([128, 1], f32)
        nc.gpsimd.memset(zt[:], 0.0)
        rmk = cp.tile([54, 1], mybir.dt.uint8)
        nc.sync.dma_start(out=rmk[:], in_=rmk_d[:])

        def bn_fold(src_d, k, nm):
            raw = cp.tile([128, 4 * k], f32, tag="bnraw" + nm)
            nc.sync.dma_start(out=raw[:], in_=src_d[:])
            s, b_, m, v = (raw[:, i * k:(i + 1) * k] for i in range(4))
            a = cp.tile([128, k], f32, tag="bna" + nm)
            c = cp.tile([128, k], f32, tag="bnc" + nm)
            t = cp.tile([128, k], f32, tag="bnt" + nm)
            nc.vector.tensor_scalar(out=t[:], in0=v, scalar1=EPS, scalar2=None,
                                    op0=OP.add)
            nc.scalar.activation(out=t[:], in_=t[:], func=AF.Sqrt,
                                 bias=zt[:, 0:1], scale=1.0)
            nc.vector.reciprocal(out=t[:], in_=t[:])
            nc.vector.tensor_tensor(out=a[:], in0=s, in1=t[:], op=OP.mult)
            nc.vector.tensor_tensor(out=t[:], in0=m, in1=a[:], op=OP.mult)
            nc.vector.tensor_tensor(out=c[:], in0=b_, in1=t[:], op=OP.subtract)
            return a, c

        a1, c1 = bn_fold(bn1_d, 4, "1")
        a2, c2 = bn_fold(bn2_d, 4, "2")
        a3, c3 = bn_fold(bn3_d, 2, "3")

        # ===== Phase 1: conv1 + BN1 + ReLU -> yT ; y2 to DRAM ; conv_off =====
        ph1 = tc.alloc_tile_pool(name="ph1", bufs=2)
        psA = tc.alloc_tile_pool(name="psA", bufs=2, space="PSUM")
        yT = ph1.tile([128, 4, NPIX], bf16, bufs=1)
        xtb = ph1.tile([128, 2, NPIX], bf16, bufs=1)
        nc.sync.dma_start(out=xtb[:, :, :], in_=xtb_d[:])
        yTs = ph1.tile([128, 4, 2176], bf16, bufs=1)
        xts = ph1.tile([128, 2, 2176], bf16, bufs=1)
        nc.sync.dma_start(out=xts[:, :, :], in_=xts_d[:])

        for nt in range(8):
            px0 = nt * 512
            for fc in range(4):
                ps = psA.tile([128, 512], f32, space="PSUM", tag="c1")
                for cc in range(2):
                    nc.tensor.matmul(
                        out=ps[:],
                        lhsT=w1[:, cc, fc * 128:(fc + 1) * 128],
                        rhs=xtb[:, cc, px0:px0 + 512],
                        start=(cc == 0), stop=(cc == 1),
                    )
                nc.scalar.activation(
                    out=yT[:, fc, px0:px0 + 512], in_=ps[:], func=AF.Relu,
                    bias=c1[:, fc:fc + 1], scale=a1[:, fc:fc + 1])

        for px0 in (0, 512, 1024, 1536, 2048):
            w_ = min(512, 2176 - px0)
            for fc in range(4):
                ps = psA.tile([128, 512], f32, space="PSUM", tag="c1")
                for cc in range(2):
                    nc.tensor.matmul(
                        out=ps[:, :w_],
                        lhsT=w1[:, cc, fc * 128:(fc + 1) * 128],
                        rhs=xts[:, cc, px0:px0 + w_],
                        start=(cc == 0), stop=(cc == 1),
                    )
                nc.scalar.activation(
                    out=yTs[:, fc, px0:px0 + w_], in_=ps[:, :w_], func=AF.Relu,
                    bias=c1[:, fc:fc + 1], scale=a1[:, fc:fc + 1])

        # y2 build: y2 row (g*4096+p), 512 elems: [0:256]=y[p,gC:], [256:512]=y[p+64,gC:]
        for grp in range(8):
            ybig = ph1.tile([128, 4, 512], bf16, tag="ybig")
            yb = ybig[:, :, :]
            for blk in range(4):
                px0 = grp * 512 + blk * 128
                pst = psA.tile([128, 512], bf16, space="PSUM", tag="ytr")
                for fc in range(4):
                    nc.tensor.transpose(
                        out=pst[:, fc * 128:(fc + 1) * 128],
                        in_=yT[:, fc, px0:px0 + 128], identity=idb[:])
                nc.scalar.copy(out=ybig[:, blk, :], in_=pst[:])
            p0 = grp * 512
            fstep = yb.ap[1][0]  # free step of dim1 (=512 elems per blk)
            for g in range(2):
                src = sap(yb, 0, 128, g * 256, [[fstep, 4], [1, 256]])
                dst = dap(y2flat, (g * 4096 + p0) * 512,
                          [[512, 128], [128 * 512, 4], [1, 256]])
                nc.gpsimd.dma_start(out=dst, in_=src)
                if p0 >= 64:
                    dst2 = dap(y2flat, (g * 4096 + p0 - 64) * 512 + 256,
                               [[512, 128], [128 * 512, 4], [1, 256]])
                    nc.gpsimd.dma_start(out=dst2, in_=src)
                else:
                    # pixels 64..127 (blk0, p>=64) -> rows 0..63
                    s3 = sap(yb, 64, 64, g * 256, [[1, 256]])
                    d3 = dap(y2flat, (g * 4096 + 0) * 512 + 256,
                             [[512, 64], [1, 256]])
                    nc.gpsimd.dma_start(out=d3, in_=s3)
                    # pixels 128..511 (blk1..3) -> rows 64..447
                    s4 = sap(yb, 0, 128, g * 256 + fstep, [[fstep, 3], [1, 256]])
                    d4 = dap(y2flat, (g * 4096 + 64) * 512 + 256,
                             [[512, 128], [128 * 512, 3], [1, 256]])
                    nc.gpsimd.dma_start(out=d4, in_=s4)

        # conv_off
        off_sb = ph1.tile([54, NP], f32, bufs=1)
        yTap = yTs[:, :, :]
        ystep = yTap.ap[1][0]
        for r0, nr in [(0, 8), (8, 8), (16, 8), (24, 6)]:
            pso = psA.tile([54, 480], f32, space="PSUM", tag="off")
            first = True
            for tap in range(9):
                dy, dx = (tap // 3) * 2, (tap % 3) * 2
                base = (r0 + dy) * 64 + dx
                for cc in range(4):
                    rhs = sap(yTap, 0, 128, cc * ystep + base,
                              [[64, nr], [1, 60]])
                    nc.tensor.matmul(
                        out=pso[:, :nr * 60],
                        lhsT=woff[:, tap * 4 + cc, :],
                        rhs=rhs,
                        start=first, stop=(tap == 8 and cc == 3),
                    )
                    first = False
            sig = ph1.tile([54, 480], f32, tag="sig")
            nc.scalar.activation(
                out=off_sb[0:54, r0 * 60:(r0 + nr) * 60], in_=pso[0:54, :nr * 60],
                func=AF.Identity, bias=bia[0:54, 0:1], scale=1.0)
            nc.scalar.activation(
                out=sig[:, :nr * 60], in_=pso[0:54, :nr * 60],
                func=AF.Sigmoid, bias=bia[0:54, 0:1], scale=1.0)
            nc.vector.copy_predicated(
                out=off_sb[0:54, r0 * 60:(r0 + nr) * 60],
                mask=rmk[:].to_broadcast([54, nr * 60]),
                data=sig[:, :nr * 60])

        offT = cp.tile([128, NT, 54], f32)
        nc.gpsimd.memset(offT[:, :, :], 0.0)
        for t in range(NT):
            px0 = t * 128
            n = min(128, NP - px0)
            pst = psA.tile([128, 64], f32, space="PSUM", tag="offtr")
            nc.tensor.transpose(out=pst[:n, 0:54], in_=off_sb[:, px0:px0 + n],
                                identity=idf[0:54, 0:54])
            nc.vector.tensor_copy(out=offT[:n, t, :], in_=pst[:n, 0:54])
        psA.release()
        ph1.release()

        # ===== Phase 2: per-sample pipeline =====
        smp = tc.alloc_tile_pool(name="smp", bufs=1)
        sh = [128, NT, 18]

        def st(tag):
            tl = smp.tile(sh, f32, tag=tag, name=tag)
            return tl[:, :, :]

        oy = offT[:, :, 0:36:2]
        ox = offT[:, :, 1:36:2]
        msk = offT[:, :, 36:54]
        tt = nc.vector.tensor_tensor
        ts = nc.vector.tensor_scalar

        def chain(base_ap, o_ap, nm):
            psh = st("psh" + nm)
            tt(out=psh, in0=base_ap, in1=o_ap, op=OP.add)
            tmp = st("tmp" + nm)
            ts(out=tmp, in0=psh, scalar1=0.5, scalar2=None, op0=OP.subtract)
            tii = smp.tile(sh, i32, tag="ti" + nm, name="ti" + nm)
            ti = tii[:, :, :]
            nc.vector.tensor_copy(out=ti, in_=tmp)
            t0f = st("t0f" + nm)
            nc.vector.tensor_copy(out=t0f, in_=ti)
            w = st("w" + nm)
            tt(out=w, in0=psh, in1=t0f, op=OP.subtract)
            bsh = st("bsh" + nm)
            ts(out=bsh, in0=t0f, scalar1=126.0, scalar2=64.0, op0=OP.min, op1=OP.max)
            inb0 = st("inb0" + nm)
            ts(out=inb0, in0=t0f, scalar1=64.0, scalar2=None, op0=OP.is_ge)
            t2 = st("t2" + nm)
            ts(out=t2, in0=t0f, scalar1=127.0, scalar2=None, op0=OP.is_le)
            tt(out=inb0, in0=inb0, in1=t2, op=OP.mult)
            inb1 = st("inb1" + nm)
            ts(out=inb1, in0=t0f, scalar1=63.0, scalar2=None, op0=OP.is_ge)
            ts(out=t2, in0=t0f, scalar1=126.0, scalar2=None, op0=OP.is_le)
            tt(out=inb1, in0=inb1, in1=t2, op=OP.mult)
            return t0f, w, bsh, inb0, inb1

        y0f, wy, bysh, yin0, yin1 = chain(pyb[:, :, :], oy, "y")
        x0f, wx, bxsh, xin0, xin1 = chain(pxb[:, :, :], ox, "x")

        def slot_weights(t0f, w, bsh, in0, in1, pfx):
            onem = st(pfx + "onem")
            ts(out=onem, in0=w, scalar1=-1.0, scalar2=1.0, op0=OP.mult, op1=OP.add)
            ta = st(pfx + "ta")
            tt(out=ta, in0=onem, in1=in0, op=OP.mult)
            tb = st(pfx + "tb")
            tt(out=tb, in0=w, in1=in1, op=OP.mult)
            eq0 = st(pfx + "eq0")
            tt(out=eq0, in0=t0f, in1=bsh, op=OP.is_equal)
            bm1 = st(pfx + "bm1")
            ts(out=bm1, in0=bsh, scalar1=1.0, scalar2=None, op0=OP.subtract)
            eqm = st(pfx + "eqm")
            tt(out=eqm, in0=t0f, in1=bm1, op=OP.is_equal)
            bp1 = st(pfx + "bp1")
            ts(out=bp1, in0=bsh, scalar1=1.0, scalar2=None, op0=OP.add)
            eqp = st(pfx + "eqp")
            tt(out=eqp, in0=t0f, in1=bp1, op=OP.is_equal)
            s0 = st(pfx + "s0")
            tt(out=s0, in0=ta, in1=eq0, op=OP.mult)
            tmp = st(pfx + "tmq")
            tt(out=tmp, in0=tb, in1=eqm, op=OP.mult)
            tt(out=s0, in0=s0, in1=tmp, op=OP.add)
            s1 = st(pfx + "s1")
            tt(out=s1, in0=ta, in1=eqp, op=OP.mult)
            tt(out=tmp, in0=tb, in1=eq0, op=OP.mult)
            tt(out=s1, in0=s1, in1=tmp, op=OP.add)
            return s0, s1

        v0, v1 = slot_weights(y0f, wy, bysh, yin0, yin1, "yv")
        u0, u1 = slot_weights(x0f, wx, bxsh, xin0, xin1, "xu")
        tt(out=v0, in0=v0, in1=msk, op=OP.mult)
        tt(out=v1, in0=v1, in1=msk, op=OP.mult)

        At = cp.tile([128, NT, 18 * 4], f32)
        Atap = At[:, :, :]
        for si, (uu, vv) in enumerate([(u0, v0), (u0, v1), (u1, v0), (u1, v1)]):
            dst = sap(Atap, 0, 128, si, [[Atap.ap[1][0], NT], [4, 18]])
            tt(out=dst, in0=uu, in1=vv, op=OP.mult)

        idxf = st("idxf")
        ts(out=idxf, in0=bysh, scalar1=32768.0, scalar2=None, op0=OP.mult)
        nc.vector.scalar_tensor_tensor(out=idxf, in0=bxsh, scalar=512.0, in1=idxf,
                                       op0=OP.mult, op1=OP.add)
        g0 = gof[:]
        gof3 = bass.AP(g0.tensor, g0.offset, [g0.ap[0], [0, NT], [1, 18]])
        tt(out=idxf, in0=idxf, in1=gof3, op=OP.add)
        idxp = cp.tile([128, NT, 18], i32)
        nc.vector.tensor_copy(out=idxp[:, :, :], in_=idxf)
        smp.release()

        # ===== Phase 3: gather / combine / flip / einsum / conv3 =====
        out3T = cp.tile([128, 2, NT * 128], f32)
        g_pool = tc.alloc_tile_pool(name="gath", bufs=2)
        v_pool = tc.alloc_tile_pool(name="valp", bufs=2)
        vt_pool = tc.alloc_tile_pool(name="valT", bufs=1)
        o2_pool = tc.alloc_tile_pool(name="o2", bufs=2)
        psB = tc.alloc_tile_pool(name="psB", bufs=2, space="PSUM")
        psE = tc.alloc_tile_pool(name="psE", bufs=1, space="PSUM")

        idxap = idxp[:, :, :]
        for n0, nn in [(0, 512), (512, 512), (1024, 512), (1536, 384)]:
            valT = vt_pool.tile([128, 36, 512], bf16, tag="vt")
            vTap = valT[:, :, :]
            for gt in range(nn // 128):
                t = n0 // 128 + gt
                val = v_pool.tile([128, 18, 256], bf16, tag="val")
                for half9 in range(3):
                    G = g_pool.tile([128, 6, 1024], bf16, tag="G")
                    for j in range(6):
                        gk = half9 * 6 + j
                        iap = sap(idxap, 0, 128, t * idxap.ap[1][0] + gk, [[1, 1]])
                        nc.gpsimd.indirect_dma_start(
                            out=G[:, j, :], out_offset=None, in_=y2col,
                            in_offset=bass.IndirectOffsetOnAxis(ap=iap, axis=0))
                    for j in range(6):
                        gk = half9 * 6 + j
                        ab = At[:, t, gk * 4:gk * 4 + 4]
                        vv = val[:, gk, :]
                        nc.vector.tensor_scalar(
                            out=vv, in0=G[:, j, 768:1024], scalar1=ab[:, 3:4],
                            scalar2=None, op0=OP.mult)
                        for s_i, lo in ((2, 512), (1, 256), (0, 0)):
                            nc.vector.scalar_tensor_tensor(
                                out=vv, in0=G[:, j, lo:lo + 256],
                                scalar=ab[:, s_i:s_i + 1], in1=vv,
                                op0=OP.mult, op1=OP.add)
                for quad in range(9):
                    pst = psB.tile([128, 512], bf16, space="PSUM", tag="vtr")
                    for b4 in range(4):
                        sl = quad * 4 + b4
                        nc.tensor.transpose(
                            out=pst[:, b4 * 128:(b4 + 1) * 128],
                            in_=val[:, sl // 2, (sl % 2) * 128:(sl % 2) * 128 + 128],
                            identity=idb[:])
                    dst = sap(vTap, 0, 128, (quad * 4) * vTap.ap[1][0] + gt * 128,
                              [[vTap.ap[1][0], 4], [1, 128]])
                    nc.scalar.copy(out=dst, in_=pst[:])

            pse = psE.tile([128, 4, 512], f32, space="PSUM", tag="e")
            for fc in range(4):
                for sl in range(36):
                    nc.tensor.matmul(
                        out=pse[:, fc, :nn],
                        lhsT=wk[:, sl, fc * 128:(fc + 1) * 128],
                        rhs=valT[:, sl, :nn],
                        start=(sl == 0), stop=(sl == 35),
                    )
            out2T = o2_pool.tile([128, 4, 512], bf16, tag="o2t")
            for fc in range(4):
                nc.scalar.activation(
                    out=out2T[:, fc, :nn], in_=pse[:, fc, :nn], func=AF.Relu,
                    bias=c2[:, fc:fc + 1], scale=a2[:, fc:fc + 1])
            ps3 = psB.tile([128, 2, 512], f32, space="PSUM", tag="c3", bufs=1)
            for cc in range(2):
                for fc in range(4):
                    nc.tensor.matmul(
                        out=ps3[:, cc, :nn],
                        lhsT=w3[:, fc, cc * 128:(cc + 1) * 128],
                        rhs=out2T[:, fc, :nn],
                        start=(fc == 0), stop=(fc == 3),
                    )
            for cc in range(2):
                nc.scalar.activation(
                    out=out3T[:, cc, n0:n0 + nn], in_=ps3[:, cc, :nn],
                    func=AF.Copy, bias=0.0, scale=a3[:, cc:cc + 1])

        for p in (psE, psB, o2_pool, vt_pool, v_pool, g_pool):
            p.release()

        # ===== Phase 4: residual + ReLU + output =====
        xres = cp.tile([128, 2, NROW * 64], f32)
        nc.sync.dma_start(out=xres[:, :, :], in_=xres_d[:])
        xbrd = cp.tile([128, 2, 128], f32)
        nc.sync.dma_start(out=xbrd[:, :, :], in_=xbrd_d[:])
        fin = tc.alloc_tile_pool(name="fin", bufs=3)
        psF = tc.alloc_tile_pool(name="psF", bufs=2, space="PSUM")
        for r in range(NROW):
            rb = fin.tile([128, 2, 64], f32, tag="rb")
            nc.vector.tensor_copy(out=rb[:, :, :],
                                  in_=xres[:, :, r * 64:(r + 1) * 64])
            for cc in range(2):
                nc.vector.scalar_tensor_tensor(
                    out=rb[:, cc, 2:62], in0=out3T[:, cc, r * 60:(r + 1) * 60],
                    scalar=c3[:, cc:cc + 1], in1=rb[:, cc, 2:62],
                    op0=OP.add, op1=OP.add)
            psf = psF.tile([64, 256], f32, space="PSUM", tag="fo")
            for cc in range(2):
                nc.tensor.transpose(out=psf[:, cc * 128:(cc + 1) * 128],
                                    in_=rb[:, cc, :], identity=idf[:])
            orow = fin.tile([64, 256], f32, tag="orow")
            nc.scalar.activation(out=orow[:], in_=psf[:], func=AF.Relu,
                                 bias=zt[0:64, 0:1], scale=1.0)
            nc.sync.dma_start(out=outA_d[r * 64:(r + 1) * 64, :], in_=orow[:])
        for r in range(2):
            psf = psF.tile([64, 256], f32, space="PSUM", tag="fo")
            for cc in range(2):
                nc.tensor.transpose(out=psf[:, cc * 128:(cc + 1) * 128],
                                    in_=xbrd[:, cc, r * 64:(r + 1) * 64],
                                    identity=idf[:])
            orow = fin.tile([64, 256], f32, tag="orow")
            nc.scalar.activation(out=orow[:], in_=psf[:], func=AF.Relu,
                                 bias=zt[0:64, 0:1], scale=1.0)
            nc.sync.dma_start(out=outB_d[r * 64:(r + 1) * 64, :], in_=orow[:])
        psF.release()
        fin.release()
        cp.release()

    nc.finalize()
    return nc


def make_inputs(core, x, w1, s1, b1, m1, v1, w_off, b_off, w_d, s2, b2, m2, v2,
                w3, s3, b3, m3, v3):
    b, half = core // 2, core % 2
    h0 = half * 30
    bfl = ml_dtypes.bfloat16
    xb = np.ascontiguousarray(x[b].reshape(4096, 256).T)       # [256, 4096]
    xtb = np.ascontiguousarray(
        xb.reshape(2, 128, 4096).transpose(1, 0, 2).reshape(128, 2 * 4096)
    ).astype(bfl)
    xs = np.ascontiguousarray(x[b][h0:h0 + 34].reshape(34 * 64, 256).T)
    xts = np.ascontiguousarray(
        xs.reshape(2, 128, 2176).transpose(1, 0, 2).reshape(128, -1)).astype(bfl)
    rows = x[b][h0 + 2: h0 + 32].reshape(NROW * 64, 256).T      # [256, 1920]
    xres = np.ascontiguousarray(
        rows.reshape(2, 128, NROW * 64).transpose(1, 0, 2).reshape(128, -1))
    brows = [0, 1] if half == 0 else [62, 63]
    bd = x[b][brows].reshape(128, 256).T
    xbrd = np.ascontiguousarray(bd.reshape(2, 128, 128).transpose(1, 0, 2)
                                .reshape(128, 256))

    w1b = np.ascontiguousarray(
        w1[0, 0].reshape(2, 128, 512).transpose(1, 0, 2).reshape(128, -1)).astype(bfl)
    wo = w_off.reshape(9, 512, 54)
    woffh = np.zeros((128, 36, 54), np.float32)
    for tap in range(9):
        for cc in range(4):
            woffh[:, tap * 4 + cc, :] = wo[tap, cc * 128:(cc + 1) * 128, :]
    woffh = woffh.reshape(128, -1).astype(bfl)
    biah = b_off.reshape(54, 1).astype(np.float32)
    wkr = w_d.reshape(9, 512, 512)
    wkh = np.zeros((128, 36, 512), np.float32)
    for g in range(2):
        for k in range(9):
            gk = g * 9 + k
            for ch in range(2):
                wkh[:, gk * 2 + ch, :] = wkr[k, g * 256 + ch * 128:
                                             g * 256 + (ch + 1) * 128, :]
    wkh = wkh.reshape(128, -1).astype(bfl)
    w3h = np.ascontiguousarray(
        w3[0, 0].reshape(4, 128, 256).transpose(1, 0, 2).reshape(128, -1)).astype(bfl)

    def bn(s, bb, m, v, k):
        out = np.zeros((128, 4 * k), np.float32)
        for i, arr in enumerate([s, bb, m, v]):
            out[:, i * k:(i + 1) * k] = np.asarray(arr).reshape(k, 128).T
        return out

    bn1 = bn(s1, b1, m1, v1, 4)
    bn2 = bn(s2, b2, m2, v2, 4)
    bn3 = bn(s3, b3, m3, v3, 2)

    pybh = np.full((128, NT, 18), 1.0e6, np.float32)
    pxbh = np.full((128, NT, 18), 1.0e6, np.float32)
    gk = np.arange(18)
    kyl = ((gk % 9) // 3) * 2.0
    kxl = ((gk % 9) % 3) * 2.0
    pix = np.arange(NP)
    hh = h0 + pix // 60
    ww = pix % 60
    for t in range(NT):
        n = min(128, NP - t * 128)
        if n > 0:
            pybh[:n, t, :] = hh[t * 128:t * 128 + n, None] + kyl[None, :] + 64.0
            pxbh[:n, t, :] = ww[t * 128:t * 128 + n, None] + kxl[None, :] + 64.0
    gofh = np.zeros((128, 18), np.float32)
    gofh[:] = ((gk // 9) * 4096 * 512 - (64 * 32768 + 64 * 512)
               ).astype(np.float32)[None, :]

    return {
        "xtb": xtb, "xts": xts, "xres": xres.astype(np.float32),
        "xbrd": xbrd.astype(np.float32),
        "w1b": w1b, "woff": woffh, "bia": biah, "wk": wkh, "w3b": w3h,
        "bn1": bn1, "bn2": bn2, "bn3": bn3,
        "pyb": pybh.reshape(128, -1), "pxb": pxbh.reshape(128, -1), "gof": gofh,
        "rmk": np.concatenate([np.zeros(36, np.uint8),
                               np.ones(18, np.uint8)]).reshape(54, 1),
        "idf": np.eye(128, dtype=np.float32),
        "idb": np.eye(128, dtype=np.float32).astype(bfl),
    }


def kernel(**inputs):
    if "nc" not in _CACHE:
        _CACHE["nc"] = build_nc()
    nc = _CACHE["nc"]
    inputs = {k: np.asarray(v) for k, v in inputs.items()}
    in_maps = [make_inputs(core, **inputs) for core in range(8)]
    res = run_bass_kernel_spmd(nc, in_maps, list(range(8)))
    out = np.zeros((4, 64, 64, 256), np.float32)
    for core in range(8):
        b, half = core // 2, core % 2
        r = res.results[core]
        oa = r["outA"].reshape(NROW, 64, 256)
        ob = r["outB"].reshape(2, 64, 256)
        out[b, half * 30 + 2: half * 30 + 32] = oa
        if half == 0:
            out[b, 0:2] = ob
        else:
            out[b, 62:64] = ob
    return out


# revision 16
# speedup vs baseline: 464.4719x; 464.4719x over previous
"""Trainium2 Bass kernel for nn_DeformSimpleBottleneck.

Sharding: 8 cores = (batch b in 0..3) x (row-half in 0..1). Each core computes
its batch's conv1 on the full 64x64 grid (needed for unrestricted deformable
sampling), then the offset conv / deformable conv / conv3 for its 30 rows of
the 60x60 inner grid, and 32 output rows (30 deform rows + 2 border rows).

Deformable bilinear sampling: a duplicated-row DRAM layout y2 (row (g,p) =
[y[p], y[p+64]] for group g's channels) lets one indirect-DMA index fetch all
4 bilinear corners as one contiguous 2KB block. Corner weights (bilinear x
mask x validity) are folded into 4 per-sample scalars applied with fused
scalar_tensor_tensor MACs; clamped indices + zero slot weights reproduce the
reference's out-of-bounds zeroing exactly.
"""
import sys
import numpy as np
import ml_dtypes

sys.path.insert(0, "/opt/trn_rl_repo")
sys.path.insert(0, "/opt/trn_rl_repo/concourse")

import concourse.bass as bass
import concourse.bacc as bacc
import concourse.mybir as mybir
import concourse.tile as tile
from concourse.bass_utils import run_bass_kernel_spmd

f32 = mybir.dt.float32
bf16 = mybir.dt.bfloat16
i32 = mybir.dt.int32
AF = mybir.ActivationFunctionType
OP = mybir.AluOpType

NPIX = 4096
F = 512
C1 = 256
NROW = 30
NP = NROW * 60     # 1800
NT = 15
EPS = 1e-5
Y2ROWS = 2 * 4096

_CACHE = {}


def sap(tile_ap, part_off, part_cnt, free_off, free_dims):
    """SBUF AP from a tile's base AP: partition offset/count + free dims."""
    pstep = tile_ap.ap[0][0]
    return bass.AP(tile_ap.tensor, tile_ap.offset + part_off * pstep + free_off,
                   [[pstep, part_cnt]] + free_dims)


def dap(base_ap, off, dims):
    """DRAM AP at element offset with explicit dims."""
    return bass.AP(base_ap.tensor, base_ap.offset + off, dims)


def build_nc():
    nc = bacc.Bacc("TRN2", target_bir_lowering=False, debug=False, num_devices=8)

    def inp(name, shape, dt=f32):
        return nc.declare_dram_parameter(name, shape, dt, isOutput=False)

    xtb_d = inp("xtb", [128, 2 * NPIX], bf16)
    xts_d = inp("xts", [128, 2 * 2176], bf16)
    xres_d = inp("xres", [128, 2 * NROW * 64], f32)
    xbrd_d = inp("xbrd", [128, 2 * 128], f32)
    w1_d = inp("w1b", [128, 2 * F], bf16)
    woff_d = inp("woff", [128, 36 * 54], bf16)
    bia_d = inp("bia", [54, 1], f32)
    wk_d = inp("wk", [128, 36 * F], bf16)
    w3_d = inp("w3b", [128, 4 * C1], bf16)
    bn1_d = inp("bn1", [128, 4 * 4])
    bn2_d = inp("bn2", [128, 4 * 4])
    bn3_d = inp("bn3", [128, 2 * 4])
    pyb_d = inp("pyb", [128, NT * 18])
    pxb_d = inp("pxb", [128, NT * 18])
    gof_d = inp("gof", [128, 18])
    rmk_d = inp("rmk", [54, 1], mybir.dt.uint8)
    idf_d = inp("idf", [128, 128])
    idb_d = inp("idb", [128, 128], bf16)
    outA_d = nc.declare_dram_parameter("outA", [NROW * 64, C1], f32, isOutput=True)
    outB_d = nc.declare_dram_parameter("outB", [2 * 64, C1], f32, isOutput=True)

    y2_d = nc.dram_tensor("y2", [Y2ROWS * 512 + 4096], bf16)
    y2col = y2_d.ap().rearrange("(a b) -> a b", b=512)
    y2flat = y2_d.ap()

    with tile.TileContext(nc) as tc:
        cp = tc.alloc_tile_pool(name="consts", bufs=1)
        w1 = cp.tile([128, 2, F], bf16)
        nc.sync.dma_start(out=w1[:, :, :], in_=w1_d[:])
        woff = cp.tile([128, 36, 54], bf16)
        nc.sync.dma_start(out=woff[:, :, :], in_=woff_d[:])
        bia = cp.tile([54, 1], f32)
        nc.sync.dma_start(out=bia[:], in_=bia_d[:])
        wk = cp.tile([128, 36, F], bf16)
        nc.sync.dma_start(out=wk[:, :, :], in_=wk_d[:])
        w3 = cp.tile([128, 4, C1], bf16)
        nc.sync.dma_start(out=w3[:, :, :], in_=w3_d[:])
        idf = cp.tile([128, 128], f32)
        nc.sync.dma_start(out=idf[:], in_=idf_d[:])
        idb = cp.tile([128, 128], bf16)
        nc.sync.dma_start(out=idb[:], in_=idb_d[:])
        pyb = cp.tile([128, NT, 18], f32)
        nc.sync.dma_start(out=pyb[:, :, :], in_=pyb_d[:])
        pxb = cp.tile([128, NT, 18], f32)
        nc.sync.dma_start(out=pxb[:, :, :], in_=pxb_d[:])
        gof = cp.tile([128, 18], f32)
        nc.sync.dma_start(out=gof[:], in_=gof_d[:])
        zt = cp.tile([128, 1], f32)
        nc.gpsimd.memset(zt[:], 0.0)
        rmk = cp.tile([54, 1], mybir.dt.uint8)
        nc.sync.dma_start(out=rmk[:], in_=rmk_d[:])

        def bn_fold(src_d, k, nm):
            raw = cp.tile([128, 4 * k], f32, tag="bnraw" + nm)
            nc.sync.dma_start(out=raw[:], in_=src_d[:])
            s, b_, m, v = (raw[:, i * k:(i + 1) * k] for i in range(4))
            a = cp.tile([128, k], f32, tag="bna" + nm)
            c = cp.tile([128, k], f32, tag="bnc" + nm)
            t = cp.tile([128, k], f32, tag="bnt" + nm)
            nc.vector.tensor_scalar(out=t[:], in0=v, scalar1=EPS, scalar2=None,
                                    op0=OP.add)
            nc.scalar.activation(out=t[:], in_=t[:], func=AF.Sqrt,
                                 bias=zt[:, 0:1], scale=1.0)
            nc.vector.reciprocal(out=t[:], in_=t[:])
            nc.vector.tensor_tensor(out=a[:], in0=s, in1=t[:], op=OP.mult)
            nc.vector.tensor_tensor(out=t[:], in0=m, in1=a[:], op=OP.mult)
            nc.vector.tensor_tensor(out=c[:], in0=b_, in1=t[:], op=OP.subtract)
            return a, c

        a1, c1 = bn_fold(bn1_d, 4, "1")
        a2, c2 = bn_fold(bn2_d, 4, "2")
        a3, c3 = bn_fold(bn3_d, 2, "3")

        # ===== Phase 1: conv1 + BN1 + ReLU -> yT ; y2 to DRAM ; conv_off =====
        ph1 = tc.alloc_tile_pool(name="ph1", bufs=2)
        psA = tc.alloc_tile_pool(name="psA", bufs=2, space="PSUM")
        yT = ph1.tile([128, 4, NPIX], bf16, bufs=1)
        xtb = ph1.tile([128, 2, NPIX], bf16, bufs=1)
        nc.sync.dma_start(out=xtb[:, :, :], in_=xtb_d[:])
        yTs = ph1.tile([128, 4, 2176], bf16, bufs=1)
        xts = ph1.tile([128, 2, 2176], bf16, bufs=1)
        nc.sync.dma_start(out=xts[:, :, :], in_=xts_d[:])

        for nt in range(8):
            px0 = nt * 512
            for fc in range(4):
                ps = psA.tile([128, 512], f32, space="PSUM", tag="c1")
                for cc in range(2):
                    nc.tensor.matmul(
                        out=ps[:],
                        lhsT=w1[:, cc, fc * 128:(fc + 1) * 128],
                        rhs=xtb[:, cc, px0:px0 + 512],
                        start=(cc == 0), stop=(cc == 1),
                    )
                nc.scalar.activation(
                    out=yT[:, fc, px0:px0 + 512], in_=ps[:], func=AF.Relu,
                    bias=c1[:, fc:fc + 1], scale=a1[:, fc:fc + 1])

        for px0 in (0, 512, 1024, 1536, 2048):
            w_ = min(512, 2176 - px0)
            for fc in range(4):
                ps = psA.tile([128, 512], f32, space="PSUM", tag="c1")
                for cc in range(2):
                    nc.tensor.matmul(
                        out=ps[:, :w_],
                        lhsT=w1[:, cc, fc * 128:(fc + 1) * 128],
                        rhs=xts[:, cc, px0:px0 + w_],
                        start=(cc == 0), stop=(cc == 1),
                    )
                nc.scalar.activation(
                    out=yTs[:, fc, px0:px0 + w_], in_=ps[:, :w_], func=AF.Relu,
                    bias=c1[:, fc:fc + 1], scale=a1[:, fc:fc + 1])

        # y2 build: y2 row (g*4096+p), 512 elems: [0:256]=y[p,gC:], [256:512]=y[p+64,gC:]
        for grp in range(8):
            ybig = ph1.tile([128, 4, 512], bf16, tag="ybig")
            yb = ybig[:, :, :]
            for blk in range(4):
                px0 = grp * 512 + blk * 128
                pst = psA.tile([128, 512], bf16, space="PSUM", tag="ytr")
                for fc in range(4):
                    nc.tensor.transpose(
                        out=pst[:, fc * 128:(fc + 1) * 128],
                        in_=yT[:, fc, px0:px0 + 128], identity=idb[:])
                nc.scalar.copy(out=ybig[:, blk, :], in_=pst[:])
            p0 = grp * 512
            fstep = yb.ap[1][0]  # free step of dim1 (=512 elems per blk)
            for g in range(2):
                src = sap(yb, 0, 128, g * 256, [[fstep, 4], [1, 256]])
                dst = dap(y2flat, (g * 4096 + p0) * 512,
                          [[512, 128], [128 * 512, 4], [1, 256]])
                nc.gpsimd.dma_start(out=dst, in_=src)
                if p0 >= 64:
                    dst2 = dap(y2flat, (g * 4096 + p0 - 64) * 512 + 256,
                               [[512, 128], [128 * 512, 4], [1, 256]])
                    nc.gpsimd.dma_start(out=dst2, in_=src)
                else:
                    # pixels 64..127 (blk0, p>=64) -> rows 0..63
                    s3 = sap(yb, 64, 64, g * 256, [[1, 256]])
                    d3 = dap(y2flat, (g * 4096 + 0) * 512 + 256,
                             [[512, 64], [1, 256]])
                    nc.gpsimd.dma_start(out=d3, in_=s3)
                    # pixels 128..511 (blk1..3) -> rows 64..447
                    s4 = sap(yb, 0, 128, g * 256 + fstep, [[fstep, 3], [1, 256]])
                    d4 = dap(y2flat, (g * 4096 + 64) * 512 + 256,
                             [[512, 128], [128 * 512, 3], [1, 256]])
                    nc.gpsimd.dma_start(out=d4, in_=s4)

        # conv_off
        off_sb = ph1.tile([54, NP], f32, bufs=1)
        yTap = yTs[:, :, :]
        ystep = yTap.ap[1][0]
        for r0, nr in [(0, 8), (8, 8), (16, 8), (24, 6)]:
            pso = psA.tile([54, 480], f32, space="PSUM", tag="off")
            first = True
            for tap in range(9):
                dy, dx = (tap // 3) * 2, (tap % 3) * 2
                base = (r0 + dy) * 64 + dx
                for cc in range(4):
                    rhs = sap(yTap, 0, 128, cc * ystep + base,
                              [[64, nr], [1, 60]])
                    nc.tensor.matmul(
                        out=pso[:, :nr * 60],
                        lhsT=woff[:, tap * 4 + cc, :],
                        rhs=rhs,
                        start=first, stop=(tap == 8 and cc == 3),
                    )
                    first = False
            sig = ph1.tile([54, 480], f32, tag="sig")
            nc.scalar.activation(
                out=off_sb[0:54, r0 * 60:(r0 + nr) * 60], in_=pso[0:54, :nr * 60],
                func=AF.Identity, bias=bia[0:54, 0:1], scale=1.0)
            nc.scalar.activation(
                out=sig[:, :nr * 60], in_=pso[0:54, :nr * 60],
                func=AF.Sigmoid, bias=bia[0:54, 0:1], scale=1.0)
            nc.vector.copy_predicated(
                out=off_sb[0:54, r0 * 60:(r0 + nr) * 60],
                mask=rmk[:].to_broadcast([54, nr * 60]),
                data=sig[:, :nr * 60])

        offT = cp.tile([128, NT, 54], f32)
        nc.gpsimd.memset(offT[:, :, :], 0.0)
        for t in range(NT):
            px0 = t * 128
            n = min(128, NP - px0)
            pst = psA.tile([128, 64], f32, space="PSUM", tag="offtr")
            nc.tensor.transpose(out=pst[:n, 0:54], in_=off_sb[:, px0:px0 + n],
                                identity=idf[0:54, 0:54])
            nc.vector.tensor_copy(out=offT[:n, t, :], in_=pst[:n, 0:54])
        psA.release()
        ph1.release()

        # ===== Phase 2: per-sample pipeline =====
        smp = tc.alloc_tile_pool(name="smp", bufs=1)
        sh = [128, NT, 18]

        def st(tag):
            tl = smp.tile(sh, f32, tag=tag, name=tag)
            return tl[:, :, :]

        oy = offT[:, :, 0:36:2]
        ox = offT[:, :, 1:36:2]
        msk = offT[:, :, 36:54]
        tt = nc.vector.tensor_tensor
        ts = nc.vector.tensor_scalar

        def chain(base_ap, o_ap, nm):
            psh = st("psh" + nm)
            tt(out=psh, in0=base_ap, in1=o_ap, op=OP.add)
            tmp = st("tmp" + nm)
            ts(out=tmp, in0=psh, scalar1=0.5, scalar2=None, op0=OP.subtract)
            tii = smp.tile(sh, i32, tag="ti" + nm, name="ti" + nm)
            ti = tii[:, :, :]
            nc.vector.tensor_copy(out=ti, in_=tmp)
            t0f = st("t0f" + nm)
            nc.vector.tensor_copy(out=t0f, in_=ti)
            w = st("w" + nm)
            tt(out=w, in0=psh, in1=t0f, op=OP.subtract)
            bsh = st("bsh" + nm)
            ts(out=bsh, in0=t0f, scalar1=126.0, scalar2=64.0, op0=OP.min, op1=OP.max)
            inb0 = st("inb0" + nm)
            ts(out=inb0, in0=t0f, scalar1=64.0, scalar2=None, op0=OP.is_ge)
            t2 = st("t2" + nm)
            ts(out=t2, in0=t0f, scalar1=127.0, scalar2=None, op0=OP.is_le)
            tt(out=inb0, in0=inb0, in1=t2, op=OP.mult)
            inb1 = st("inb1" + nm)
            ts(out=inb1, in0=t0f, scalar1=63.0, scalar2=None, op0=OP.is_ge)
            ts(out=t2, in0=t0f, scalar1=126.0, scalar2=None, op0=OP.is_le)
            tt(out=inb1, in0=inb1, in1=t2, op=OP.mult)
            return t0f, w, bsh, inb0, inb1

        y0f, wy, bysh, yin0, yin1 = chain(pyb[:, :, :], oy, "y")
        x0f, wx, bxsh, xin0, xin1 = chain(pxb[:, :, :], ox, "x")

        def slot_weights(t0f, w, bsh, in0, in1, pfx):
            onem = st(pfx + "onem")
            ts(out=onem, in0=w, scalar1=-1.0, scalar2=1.0, op0=OP.mult, op1=OP.add)
            ta = st(pfx + "ta")
            tt(out=ta, in0=onem, in1=in0, op=OP.mult)
            tb = st(pfx + "tb")
            tt(out=tb, in0=w, in1=in1, op=OP.mult)
            eq0 = st(pfx + "eq0")
            tt(out=eq0, in0=t0f, in1=bsh, op=OP.is_equal)
            bm1 = st(pfx + "bm1")
            ts(out=bm1, in0=bsh, scalar1=1.0, scalar2=None, op0=OP.subtract)
            eqm = st(pfx + "eqm")
            tt(out=eqm, in0=t0f, in1=bm1, op=OP.is_equal)
            bp1 = st(pfx + "bp1")
            ts(out=bp1, in0=bsh, scalar1=1.0, scalar2=None, op0=OP.add)
            eqp = st(pfx + "eqp")
            tt(out=eqp, in0=t0f, in1=bp1, op=OP.is_equal)
            s0 = st(pfx + "s0")
            tt(out=s0, in0=ta, in1=eq0, op=OP.mult)
            tmp = st(pfx + "tmq")
            tt(out=tmp, in0=tb, in1=eqm, op=OP.mult)
            tt(out=s0, in0=s0, in1=tmp, op=OP.add)
            s1 = st(pfx + "s1")
            tt(out=s1, in0=ta, in1=eqp, op=OP.mult)
            tt(out=tmp, in0=tb, in1=eq0, op=OP.mult)
            tt(out=s1, in0=s1, in1=tmp, op=OP.add)
            return s0, s1

        v0, v1 = slot_weights(y0f, wy, bysh, yin0, yin1, "yv")
        u0, u1 = slot_weights(x0f, wx, bxsh, xin0, xin1, "xu")
        tt(out=v0, in0=v0, in1=msk, op=OP.mult)
        tt(out=v1, in0=v1, in1=msk, op=OP.mult)

        At = cp.tile([128, NT, 18 * 4], f32)
        Atap = At[:, :, :]
        for si, (uu, vv) in enumerate([(u0, v0), (u0, v1), (u1, v0), (u1, v1)]):
            dst = sap(Atap, 0, 128, si, [[Atap.ap[1][0], NT], [4, 18]])
            tt(out=dst, in0=uu, in1=vv, op=OP.mult)

        idxf = st("idxf")
        ts(out=idxf, in0=bysh, scalar1=64.0, scalar2=None, op0=OP.mult)
        nc.vector.tensor_tensor(out=idxf, in0=bxsh, in1=idxf, op=OP.add)
        g0 = gof[:]
        gof3 = bass.AP(g0.tensor, g0.offset, [g0.ap[0], [0, NT], [1, 18]])
        tt(out=idxf, in0=idxf, in1=gof3, op=OP.add)
        idxp = cp.tile([128, NT, 18], i32)
        nc.vector.tensor_copy(out=idxp[:, :, :], in_=idxf)
        smp.release()

        # ===== Phase 3: gather / combine / flip / einsum / conv3 =====
        out3T = cp.tile([128, 2, NT * 128], f32)
        g_pool = tc.alloc_tile_pool(name="gath", bufs=2)
        v_pool = tc.alloc_tile_pool(name="valp", bufs=2)
        vt_pool = tc.alloc_tile_pool(name="valT", bufs=1)
        o2_pool = tc.alloc_tile_pool(name="o2", bufs=2)
        psB = tc.alloc_tile_pool(name="psB", bufs=2, space="PSUM")
        psE = tc.alloc_tile_pool(name="psE", bufs=1, space="PSUM")

        idxap = idxp[:, :, :]
        for n0, nn in [(0, 512), (512, 512), (1024, 512), (1536, 384)]:
            valT = vt_pool.tile([128, 36, 512], bf16, tag="vt")
            vTap = valT[:, :, :]
            for gt in range(nn // 128):
                t = n0 // 128 + gt
                val = v_pool.tile([128, 18, 256], bf16, tag="val")
                for half9 in range(3):
                    G = g_pool.tile([128, 6, 1024], bf16, tag="G")
                    for j in range(6):
                        gk = half9 * 6 + j
                        iap = sap(idxap, 0, 128, t * idxap.ap[1][0] + gk, [[1, 1]])
                        nc.gpsimd.indirect_dma_start(
                            out=G[:, j, :], out_offset=None, in_=y2col,
                            in_offset=bass.IndirectOffsetOnAxis(ap=iap, axis=0))
                    for j in range(6):
                        gk = half9 * 6 + j
                        ab = At[:, t, gk * 4:gk * 4 + 4]
                        vv = val[:, gk, :]
                        nc.vector.tensor_scalar(
                            out=vv, in0=G[:, j, 768:1024], scalar1=ab[:, 3:4],
                            scalar2=None, op0=OP.mult)
                        for s_i, lo in ((2, 512), (1, 256), (0, 0)):
                            nc.vector.scalar_tensor_tensor(
                                out=vv, in0=G[:, j, lo:lo + 256],
                                scalar=ab[:, s_i:s_i + 1], in1=vv,
                                op0=OP.mult, op1=OP.add)
                for quad in range(9):
                    pst = psB.tile([128, 512], bf16, space="PSUM", tag="vtr")
                    for b4 in range(4):
                        sl = quad * 4 + b4
                        nc.tensor.transpose(
                            out=pst[:, b4 * 128:(b4 + 1) * 128],
                            in_=val[:, sl // 2, (sl % 2) * 128:(sl % 2) * 128 + 128],
                            identity=idb[:])
                    dst = sap(vTap, 0, 128, (quad * 4) * vTap.ap[1][0] + gt * 128,
                              [[vTap.ap[1][0], 4], [1, 128]])
                    nc.scalar.copy(out=dst, in_=pst[:])

            pse = psE.tile([128, 4, 512], f32, space="PSUM", tag="e")
            for fc in range(4):
                for sl in range(36):
                    nc.tensor.matmul(
                        out=pse[:, fc, :nn],
                        lhsT=wk[:, sl, fc * 128:(fc + 1) * 128],
                        rhs=valT[:, sl, :nn],
                        start=(sl == 0), stop=(sl == 35),
                    )
            out2T = o2_pool.tile([128, 4, 512], bf16, tag="o2t")
            for fc in range(4):
                nc.scalar.activation(
                    out=out2T[:, fc, :nn], in_=pse[:, fc, :nn], func=AF.Relu,
                    bias=c2[:, fc:fc + 1], scale=a2[:, fc:fc + 1])
            ps3 = psB.tile([128, 2, 512], f32, space="PSUM", tag="c3", bufs=1)
            for cc in range(2):
                for fc in range(4):
                    nc.tensor.matmul(
                        out=ps3[:, cc, :nn],
                        lhsT=w3[:, fc, cc * 128:(cc + 1) * 128],
                        rhs=out2T[:, fc, :nn],
                        start=(fc == 0), stop=(fc == 3),
                    )
            for cc in range(2):
                nc.scalar.activation(
                    out=out3T[:, cc, n0:n0 + nn], in_=ps3[:, cc, :nn],
                    func=AF.Copy, bias=0.0, scale=a3[:, cc:cc + 1])

        for p in (psE, psB, o2_pool, vt_pool, v_pool, g_pool):
            p.release()

        # ===== Phase 4: residual + ReLU + output =====
        xres = cp.tile([128, 2, NROW * 64], f32)
        nc.sync.dma_start(out=xres[:, :, :], in_=xres_d[:])
        xbrd = cp.tile([128, 2, 128], f32)
        nc.sync.dma_start(out=xbrd[:, :, :], in_=xbrd_d[:])
        fin = tc.alloc_tile_pool(name="fin", bufs=3)
        psF = tc.alloc_tile_pool(name="psF", bufs=2, space="PSUM")
        for r in range(NROW):
            rb = fin.tile([128, 2, 64], f32, tag="rb")
            nc.vector.tensor_copy(out=rb[:, :, :],
                                  in_=xres[:, :, r * 64:(r + 1) * 64])
            for cc in range(2):
                nc.vector.scalar_tensor_tensor(
                    out=rb[:, cc, 2:62], in0=out3T[:, cc, r * 60:(r + 1) * 60],
                    scalar=c3[:, cc:cc + 1], in1=rb[:, cc, 2:62],
                    op0=OP.add, op1=OP.add)
            psf = psF.tile([64, 256], f32, space="PSUM", tag="fo")
            for cc in range(2):
                nc.tensor.transpose(out=psf[:, cc * 128:(cc + 1) * 128],
                                    in_=rb[:, cc, :], identity=idf[:])
            orow = fin.tile([64, 256], f32, tag="orow")
            nc.scalar.activation(out=orow[:], in_=psf[:], func=AF.Relu,
                                 bias=zt[0:64, 0:1], scale=1.0)
            nc.sync.dma_start(out=outA_d[r * 64:(r + 1) * 64, :], in_=orow[:])
        for r in range(2):
            psf = psF.tile([64, 256], f32, space="PSUM", tag="fo")
            for cc in range(2):
                nc.tensor.transpose(out=psf[:, cc * 128:(cc + 1) * 128],
                                    in_=xbrd[:, cc, r * 64:(r + 1) * 64],
                                    identity=idf[:])
            orow = fin.tile([64, 256], f32, tag="orow")
            nc.scalar.activation(out=orow[:], in_=psf[:], func=AF.Relu,
                                 bias=zt[0:64, 0:1], scale=1.0)
            nc.sync.dma_start(out=outB_d[r * 64:(r + 1) * 64, :], in_=orow[:])
        psF.release()
        fin.release()
        cp.release()

    nc.finalize()
    return nc


def make_inputs(core, x, w1, s1, b1, m1, v1, w_off, b_off, w_d, s2, b2, m2, v2,
                w3, s3, b3, m3, v3):
    b, half = core // 2, core % 2
    h0 = half * 30
    bfl = ml_dtypes.bfloat16
    xb = np.ascontiguousarray(x[b].reshape(4096, 256).T)       # [256, 4096]
    xtb = np.ascontiguousarray(
        xb.reshape(2, 128, 4096).transpose(1, 0, 2).reshape(128, 2 * 4096)
    ).astype(bfl)
    xs = np.ascontiguousarray(x[b][h0:h0 + 34].reshape(34 * 64, 256).T)
    xts = np.ascontiguousarray(
        xs.reshape(2, 128, 2176).transpose(1, 0, 2).reshape(128, -1)).astype(bfl)
    rows = x[b][h0 + 2: h0 + 32].reshape(NROW * 64, 256).T      # [256, 1920]
    xres = np.ascontiguousarray(
        rows.reshape(2, 128, NROW * 64).transpose(1, 0, 2).reshape(128, -1))
    brows = [0, 1] if half == 0 else [62, 63]
    bd = x[b][brows].reshape(128, 256).T
    xbrd = np.ascontiguousarray(bd.reshape(2, 128, 128).transpose(1, 0, 2)
                                .reshape(128, 256))

    w1b = np.ascontiguousarray(
        w1[0, 0].reshape(2, 128, 512).transpose(1, 0, 2).reshape(128, -1)).astype(bfl)
    wo = w_off.reshape(9, 512, 54)
    woffh = np.zeros((128, 36, 54), np.float32)
    for tap in range(9):
        for cc in range(4):
            woffh[:, tap * 4 + cc, :] = wo[tap, cc * 128:(cc + 1) * 128, :]
    woffh = woffh.reshape(128, -1).astype(bfl)
    biah = b_off.reshape(54, 1).astype(np.float32)
    wkr = w_d.reshape(9, 512, 512)
    wkh = np.zeros((128, 36, 512), np.float32)
    for g in range(2):
        for k in range(9):
            gk = g * 9 + k
            for ch in range(2):
                wkh[:, gk * 2 + ch, :] = wkr[k, g * 256 + ch * 128:
                                             g * 256 + (ch + 1) * 128, :]
    wkh = wkh.reshape(128, -1).astype(bfl)
    w3h = np.ascontiguousarray(
        w3[0, 0].reshape(4, 128, 256).transpose(1, 0, 2).reshape(128, -1)).astype(bfl)

    def bn(s, bb, m, v, k):
        out = np.zeros((128, 4 * k), np.float32)
        for i, arr in enumerate([s, bb, m, v]):
            out[:, i * k:(i + 1) * k] = np.asarray(arr).reshape(k, 128).T
        return out

    bn1 = bn(s1, b1, m1, v1, 4)
    bn2 = bn(s2, b2, m2, v2, 4)
    bn3 = bn(s3, b3, m3, v3, 2)

    pybh = np.full((128, NT, 18), 1.0e6, np.float32)
    pxbh = np.full((128, NT, 18), 1.0e6, np.float32)
    gk = np.arange(18)
    kyl = ((gk % 9) // 3) * 2.0
    kxl = ((gk % 9) % 3) * 2.0
    pix = np.arange(NP)
    hh = h0 + pix // 60
    ww = pix % 60
    for t in range(NT):
        n = min(128, NP - t * 128)
        if n > 0:
            pybh[:n, t, :] = hh[t * 128:t * 128 + n, None] + kyl[None, :] + 64.0
            pxbh[:n, t, :] = ww[t * 128:t * 128 + n, None] + kxl[None, :] + 64.0
    gofh = np.zeros((128, 18), np.float32)
    gofh[:] = ((gk // 9) * 4096 - (64 * 64 + 64)).astype(np.float32)[None, :]

    return {
        "xtb": xtb, "xts": xts, "xres": xres.astype(np.float32),
        "xbrd": xbrd.astype(np.float32),
        "w1b": w1b, "woff": woffh, "bia": biah, "wk": wkh, "w3b": w3h,
        "bn1": bn1, "bn2": bn2, "bn3": bn3,
        "pyb": pybh.reshape(128, -1), "pxb": pxbh.reshape(128, -1), "gof": gofh,
        "rmk": np.concatenate([np.zeros(36, np.uint8),
                               np.ones(18, np.uint8)]).reshape(54, 1),
        "idf": np.eye(128, dtype=np.float32),
        "idb": np.eye(128, dtype=np.float32).astype(bfl),
    }


def kernel(**inputs):
    if "nc" not in _CACHE:
        _CACHE["nc"] = build_nc()
    nc = _CACHE["nc"]
    inputs = {k: np.asarray(v) for k, v in inputs.items()}
    in_maps = [make_inputs(core, **inputs) for core in range(8)]
    res = run_bass_kernel_spmd(nc, in_maps, list(range(8)))
    out = np.zeros((4, 64, 64, 256), np.float32)
    for core in range(8):
        b, half = core // 2, core % 2
        r = res.results[core]
        oa = r["outA"].reshape(NROW, 64, 256)
        ob = r["outB"].reshape(2, 64, 256)
        out[b, half * 30 + 2: half * 30 + 32] = oa
        if half == 0:
            out[b, 0:2] = ob
        else:
            out[b, 62:64] = ob
    return out


# revision 21
# speedup vs baseline: 475.8759x; 1.0246x over previous
"""Trainium2 Bass kernel for nn_DeformSimpleBottleneck.

Sharding: 8 cores = (batch b in 0..3) x (row-half in 0..1). Each core computes
its batch's conv1 on the full 64x64 grid (needed for unrestricted deformable
sampling), then the offset conv / deformable conv / conv3 for its 30 rows of
the 60x60 inner grid, and 32 output rows (30 deform rows + 2 border rows).

Deformable bilinear sampling: a duplicated-row DRAM layout y2 (row (g,p) =
[y[p], y[p+64]] for group g's channels) lets one indirect-DMA index fetch all
4 bilinear corners as one contiguous 2KB block. Corner weights (bilinear x
mask x validity) are folded into 4 per-sample scalars applied with fused
scalar_tensor_tensor MACs; clamped indices + zero slot weights reproduce the
reference's out-of-bounds zeroing exactly.
"""
import sys
import numpy as np
import ml_dtypes

sys.path.insert(0, "/opt/trn_rl_repo")
sys.path.insert(0, "/opt/trn_rl_repo/concourse")

import concourse.bass as bass
import concourse.bacc as bacc
import concourse.mybir as mybir
import concourse.tile as tile
from concourse.bass_utils import run_bass_kernel_spmd

f32 = mybir.dt.float32
bf16 = mybir.dt.bfloat16
i32 = mybir.dt.int32
AF = mybir.ActivationFunctionType
OP = mybir.AluOpType

NPIX = 4096
F = 512
C1 = 256
NROW = 30
NP = NROW * 60     # 1800
NT = 15
EPS = 1e-5
Y2ROWS = 2 * 4096

_CACHE = {}


def sap(tile_ap, part_off, part_cnt, free_off, free_dims):
    """SBUF AP from a tile's base AP: partition offset/count + free dims."""
    pstep = tile_ap.ap[0][0]
    return bass.AP(tile_ap.tensor, tile_ap.offset + part_off * pstep + free_off,
                   [[pstep, part_cnt]] + free_dims)


def dap(base_ap, off, dims):
    """DRAM AP at element offset with explicit dims."""
    return bass.AP(base_ap.tensor, base_ap.offset + off, dims)


def build_nc():
    nc = bacc.Bacc("TRN2", target_bir_lowering=False, debug=False, num_devices=8)

    def inp(name, shape, dt=f32):
        return nc.declare_dram_parameter(name, shape, dt, isOutput=False)

    xtb_d = inp("xtb", [128, 2 * NPIX], bf16)
    xts_d = inp("xts", [128, 2 * 2176], bf16)
    xres_d = inp("xres", [128, 2 * NROW * 64], f32)
    xbrd_d = inp("xbrd", [128, 2 * 128], f32)
    w1_d = inp("w1b", [128, 2 * F], bf16)
    woff_d = inp("woff", [128, 36 * 54], bf16)
    bia_d = inp("bia", [54, 1], f32)
    wk_d = inp("wk", [128, 36 * F], bf16)
    w3_d = inp("w3b", [128, 4 * C1], bf16)
    bn1_d = inp("bn1", [128, 4 * 4])
    bn2_d = inp("bn2", [128, 4 * 4])
    bn3_d = inp("bn3", [128, 2 * 4])
    pyb_d = inp("pyb", [128, NT * 18])
    pxb_d = inp("pxb", [128, NT * 18])
    gof_d = inp("gof", [128, 18])
    rmk_d = inp("rmk", [54, 1], mybir.dt.uint8)
    idf_d = inp("idf", [128, 128])
    idb_d = inp("idb", [128, 128], bf16)
    outA_d = nc.declare_dram_parameter("outA", [NROW * 64, C1], f32, isOutput=True)
    outB_d = nc.declare_dram_parameter("outB", [2 * 64, C1], f32, isOutput=True)

    y2_d = nc.dram_tensor("y2", [Y2ROWS * 512 + 4096], bf16)
    y2col = y2_d.ap().rearrange("(a b) -> a b", b=512)
    y2flat = y2_d.ap()

    with tile.TileContext(nc) as tc:
        cp = tc.alloc_tile_pool(name="consts", bufs=1)
        w1 = cp.tile([128, 2, F], bf16)
        nc.sync.dma_start(out=w1[:, :, :], in_=w1_d[:])
        woff = cp.tile([128, 36, 54], bf16)
        nc.sync.dma_start(out=woff[:, :, :], in_=woff_d[:])
        bia = cp.tile([54, 1], f32)
        nc.sync.dma_start(out=bia[:], in_=bia_d[:])
        wk = cp.tile([128, 36, F], bf16)
        nc.sync.dma_start(out=wk[:, :, :], in_=wk_d[:])
        w3 = cp.tile([128, 4, C1], bf16)
        nc.sync.dma_start(out=w3[:, :, :], in_=w3_d[:])
        idf = cp.tile([128, 128], f32)
        nc.sync.dma_start(out=idf[:], in_=idf_d[:])
        idb = cp.tile([128, 128], bf16)
        nc.sync.dma_start(out=idb[:], in_=idb_d[:])
        pyb = cp.tile([128, NT, 18], f32)
        nc.sync.dma_start(out=pyb[:, :, :], in_=pyb_d[:])
        pxb = cp.tile([128, NT, 18], f32)
        nc.sync.dma_start(out=pxb[:, :, :], in_=pxb_d[:])
        gof = cp.tile([128, 18], f32)
        nc.sync.dma_start(out=gof[:], in_=gof_d[:])
        zt = cp.tile([128, 1], f32)
        nc.gpsimd.memset(zt[:], 0.0)
        xres = cp.tile([128, 2, NROW * 64], f32)
        nc.sync.dma_start(out=xres[:, :, :], in_=xres_d[:])
        xbrd = cp.tile([128, 2, 128], f32)
        nc.sync.dma_start(out=xbrd[:, :, :], in_=xbrd_d[:])
        rmk = cp.tile([54, 1], mybir.dt.uint8)
        nc.sync.dma_start(out=rmk[:], in_=rmk_d[:])

        def bn_fold(src_d, k, nm):
            raw = cp.tile([128, 4 * k], f32, tag="bnraw" + nm)
            nc.sync.dma_start(out=raw[:], in_=src_d[:])
            s, b_, m, v = (raw[:, i * k:(i + 1) * k] for i in range(4))
            a = cp.tile([128, k], f32, tag="bna" + nm)
            c = cp.tile([128, k], f32, tag="bnc" + nm)
            t = cp.tile([128, k], f32, tag="bnt" + nm)
            nc.vector.tensor_scalar(out=t[:], in0=v, scalar1=EPS, scalar2=None,
                                    op0=OP.add)
            nc.scalar.activation(out=t[:], in_=t[:], func=AF.Sqrt,
                                 bias=zt[:, 0:1], scale=1.0)
            nc.vector.reciprocal(out=t[:], in_=t[:])
            nc.vector.tensor_tensor(out=a[:], in0=s, in1=t[:], op=OP.mult)
            nc.vector.tensor_tensor(out=t[:], in0=m, in1=a[:], op=OP.mult)
            nc.vector.tensor_tensor(out=c[:], in0=b_, in1=t[:], op=OP.subtract)
            return a, c

        a1, c1 = bn_fold(bn1_d, 4, "1")
        a2, c2 = bn_fold(bn2_d, 4, "2")
        a3, c3 = bn_fold(bn3_d, 2, "3")

        # ===== Phase 1: conv1 + BN1 + ReLU -> yT ; y2 to DRAM ; conv_off =====
        ph1 = tc.alloc_tile_pool(name="ph1", bufs=2)
        psA = tc.alloc_tile_pool(name="psA", bufs=2, space="PSUM")
        yT = ph1.tile([128, 4, NPIX], bf16, bufs=1)
        xtb = ph1.tile([128, 2, NPIX], bf16, bufs=1)
        nc.sync.dma_start(out=xtb[:, :, :], in_=xtb_d[:])
        yTs = ph1.tile([128, 4, 2176], bf16, bufs=1)
        xts = ph1.tile([128, 2, 2176], bf16, bufs=1)
        nc.sync.dma_start(out=xts[:, :, :], in_=xts_d[:])

        for nt in range(8):
            px0 = nt * 512
            for fc in range(4):
                ps = psA.tile([128, 512], f32, space="PSUM", tag="c1")
                for cc in range(2):
                    nc.tensor.matmul(
                        out=ps[:],
                        lhsT=w1[:, cc, fc * 128:(fc + 1) * 128],
                        rhs=xtb[:, cc, px0:px0 + 512],
                        start=(cc == 0), stop=(cc == 1),
                    )
                nc.scalar.activation(
                    out=yT[:, fc, px0:px0 + 512], in_=ps[:], func=AF.Relu,
                    bias=c1[:, fc:fc + 1], scale=a1[:, fc:fc + 1])

        for px0 in (0, 512, 1024, 1536, 2048):
            w_ = min(512, 2176 - px0)
            for fc in range(4):
                ps = psA.tile([128, 512], f32, space="PSUM", tag="c1")
                for cc in range(2):
                    nc.tensor.matmul(
                        out=ps[:, :w_],
                        lhsT=w1[:, cc, fc * 128:(fc + 1) * 128],
                        rhs=xts[:, cc, px0:px0 + w_],
                        start=(cc == 0), stop=(cc == 1),
                    )
                nc.scalar.activation(
                    out=yTs[:, fc, px0:px0 + w_], in_=ps[:, :w_], func=AF.Relu,
                    bias=c1[:, fc:fc + 1], scale=a1[:, fc:fc + 1])

        # y2 build: y2 row (g*4096+p), 512 elems: [0:256]=y[p,gC:], [256:512]=y[p+64,gC:]
        for grp in range(8):
            ybig = ph1.tile([128, 4, 512], bf16, tag="ybig")
            yb = ybig[:, :, :]
            for blk in range(4):
                px0 = grp * 512 + blk * 128
                pst = psA.tile([128, 512], bf16, space="PSUM", tag="ytr")
                for fc in range(4):
                    nc.tensor.transpose(
                        out=pst[:, fc * 128:(fc + 1) * 128],
                        in_=yT[:, fc, px0:px0 + 128], identity=idb[:])
                nc.scalar.copy(out=ybig[:, blk, :], in_=pst[:])
            p0 = grp * 512
            fstep = yb.ap[1][0]  # free step of dim1 (=512 elems per blk)
            for g in range(2):
                src = sap(yb, 0, 128, g * 256, [[fstep, 4], [1, 256]])
                dst = dap(y2flat, (g * 4096 + p0) * 512,
                          [[512, 128], [128 * 512, 4], [1, 256]])
                nc.sync.dma_start(out=dst, in_=src)
                if p0 >= 64:
                    dst2 = dap(y2flat, (g * 4096 + p0 - 64) * 512 + 256,
                               [[512, 128], [128 * 512, 4], [1, 256]])
                    nc.sync.dma_start(out=dst2, in_=src)
                else:
                    # pixels 64..127 (blk0, p>=64) -> rows 0..63
                    s3 = sap(yb, 64, 64, g * 256, [[1, 256]])
                    d3 = dap(y2flat, (g * 4096 + 0) * 512 + 256,
                             [[512, 64], [1, 256]])
                    nc.sync.dma_start(out=d3, in_=s3)
                    # pixels 128..511 (blk1..3) -> rows 64..447
                    s4 = sap(yb, 0, 128, g * 256 + fstep, [[fstep, 3], [1, 256]])
                    d4 = dap(y2flat, (g * 4096 + 64) * 512 + 256,
                             [[512, 128], [128 * 512, 3], [1, 256]])
                    nc.sync.dma_start(out=d4, in_=s4)

        # conv_off
        off_sb = ph1.tile([54, NP], f32, bufs=1)
        yTap = yTs[:, :, :]
        ystep = yTap.ap[1][0]
        for r0, nr in [(0, 8), (8, 8), (16, 8), (24, 6)]:
            pso = psA.tile([54, 480], f32, space="PSUM", tag="off")
            first = True
            for tap in range(9):
                dy, dx = (tap // 3) * 2, (tap % 3) * 2
                base = (r0 + dy) * 64 + dx
                for cc in range(4):
                    rhs = sap(yTap, 0, 128, cc * ystep + base,
                              [[64, nr], [1, 60]])
                    nc.tensor.matmul(
                        out=pso[:, :nr * 60],
                        lhsT=woff[:, tap * 4 + cc, :],
                        rhs=rhs,
                        start=first, stop=(tap == 8 and cc == 3),
                    )
                    first = False
            sig = ph1.tile([54, 480], f32, tag="sig")
            nc.scalar.activation(
                out=off_sb[0:54, r0 * 60:(r0 + nr) * 60], in_=pso[0:54, :nr * 60],
                func=AF.Identity, bias=bia[0:54, 0:1], scale=1.0)
            nc.scalar.activation(
                out=sig[:, :nr * 60], in_=pso[0:54, :nr * 60],
                func=AF.Sigmoid, bias=bia[0:54, 0:1], scale=1.0)
            nc.vector.copy_predicated(
                out=off_sb[0:54, r0 * 60:(r0 + nr) * 60],
                mask=rmk[:].to_broadcast([54, nr * 60]),
                data=sig[:, :nr * 60])

        offT = cp.tile([128, NT, 54], f32)
        nc.gpsimd.memset(offT[:, :, :], 0.0)
        for t in range(NT):
            px0 = t * 128
            n = min(128, NP - px0)
            pst = psA.tile([128, 64], f32, space="PSUM", tag="offtr")
            nc.tensor.transpose(out=pst[:n, 0:54], in_=off_sb[:, px0:px0 + n],
                                identity=idf[0:54, 0:54])
            nc.vector.tensor_copy(out=offT[:n, t, :], in_=pst[:n, 0:54])
        psA.release()
        ph1.release()

        # ===== Phase 2: per-sample pipeline =====
        smp = tc.alloc_tile_pool(name="smp", bufs=1)
        sh = [128, NT, 18]

        def st(tag):
            tl = smp.tile(sh, f32, tag=tag, name=tag)
            return tl[:, :, :]

        oy = offT[:, :, 0:36:2]
        ox = offT[:, :, 1:36:2]
        msk = offT[:, :, 36:54]
        tt = nc.vector.tensor_tensor
        ts = nc.vector.tensor_scalar

        def chain(base_ap, o_ap, nm):
            psh = st("psh" + nm)
            tt(out=psh, in0=base_ap, in1=o_ap, op=OP.add)
            tmp = st("tmp" + nm)
            ts(out=tmp, in0=psh, scalar1=0.5, scalar2=None, op0=OP.subtract)
            tii = smp.tile(sh, i32, tag="ti" + nm, name="ti" + nm)
            ti = tii[:, :, :]
            nc.vector.tensor_copy(out=ti, in_=tmp)
            t0f = st("t0f" + nm)
            nc.vector.tensor_copy(out=t0f, in_=ti)
            w = st("w" + nm)
            tt(out=w, in0=psh, in1=t0f, op=OP.subtract)
            bsh = st("bsh" + nm)
            ts(out=bsh, in0=t0f, scalar1=126.0, scalar2=64.0, op0=OP.min, op1=OP.max)
            inb0 = st("inb0" + nm)
            ts(out=inb0, in0=t0f, scalar1=64.0, scalar2=None, op0=OP.is_ge)
            t2 = st("t2" + nm)
            ts(out=t2, in0=t0f, scalar1=127.0, scalar2=None, op0=OP.is_le)
            tt(out=inb0, in0=inb0, in1=t2, op=OP.mult)
            inb1 = st("inb1" + nm)
            ts(out=inb1, in0=t0f, scalar1=63.0, scalar2=None, op0=OP.is_ge)
            ts(out=t2, in0=t0f, scalar1=126.0, scalar2=None, op0=OP.is_le)
            tt(out=inb1, in0=inb1, in1=t2, op=OP.mult)
            return t0f, w, bsh, inb0, inb1

        y0f, wy, bysh, yin0, yin1 = chain(pyb[:, :, :], oy, "y")
        x0f, wx, bxsh, xin0, xin1 = chain(pxb[:, :, :], ox, "x")

        def slot_weights(t0f, w, bsh, in0, in1, pfx):
            onem = st(pfx + "onem")
            ts(out=onem, in0=w, scalar1=-1.0, scalar2=1.0, op0=OP.mult, op1=OP.add)
            ta = st(pfx + "ta")
            tt(out=ta, in0=onem, in1=in0, op=OP.mult)
            tb = st(pfx + "tb")
            tt(out=tb, in0=w, in1=in1, op=OP.mult)
            eq0 = st(pfx + "eq0")
            tt(out=eq0, in0=t0f, in1=bsh, op=OP.is_equal)
            bm1 = st(pfx + "bm1")
            ts(out=bm1, in0=bsh, scalar1=1.0, scalar2=None, op0=OP.subtract)
            eqm = st(pfx + "eqm")
            tt(out=eqm, in0=t0f, in1=bm1, op=OP.is_equal)
            bp1 = st(pfx + "bp1")
            ts(out=bp1, in0=bsh, scalar1=1.0, scalar2=None, op0=OP.add)
            eqp = st(pfx + "eqp")
            tt(out=eqp, in0=t0f, in1=bp1, op=OP.is_equal)
            s0 = st(pfx + "s0")
            tt(out=s0, in0=ta, in1=eq0, op=OP.mult)
            tmp = st(pfx + "tmq")
            tt(out=tmp, in0=tb, in1=eqm, op=OP.mult)
            tt(out=s0, in0=s0, in1=tmp, op=OP.add)
            s1 = st(pfx + "s1")
            tt(out=s1, in0=ta, in1=eqp, op=OP.mult)
            tt(out=tmp, in0=tb, in1=eq0, op=OP.mult)
            tt(out=s1, in0=s1, in1=tmp, op=OP.add)
            return s0, s1

        v0, v1 = slot_weights(y0f, wy, bysh, yin0, yin1, "yv")
        u0, u1 = slot_weights(x0f, wx, bxsh, xin0, xin1, "xu")
        tt(out=v0, in0=v0, in1=msk, op=OP.mult)
        tt(out=v1, in0=v1, in1=msk, op=OP.mult)

        At = cp.tile([128, NT, 18 * 4], f32)
        Atap = At[:, :, :]
        for si, (uu, vv) in enumerate([(u0, v0), (u0, v1), (u1, v0), (u1, v1)]):
            dst = sap(Atap, 0, 128, si, [[Atap.ap[1][0], NT], [4, 18]])
            tt(out=dst, in0=uu, in1=vv, op=OP.mult)

        idxf = st("idxf")
        ts(out=idxf, in0=bysh, scalar1=64.0, scalar2=None, op0=OP.mult)
        nc.vector.tensor_tensor(out=idxf, in0=bxsh, in1=idxf, op=OP.add)
        g0 = gof[:]
        gof3 = bass.AP(g0.tensor, g0.offset, [g0.ap[0], [0, NT], [1, 18]])
        tt(out=idxf, in0=idxf, in1=gof3, op=OP.add)
        idxp = cp.tile([128, NT, 18], i32)
        nc.vector.tensor_copy(out=idxp[:, :, :], in_=idxf)
        smp.release()

        # ===== Phase 3: gather / combine / flip / einsum / conv3 =====
        out3T = cp.tile([128, 2, NT * 128], f32)
        fin = tc.alloc_tile_pool(name="fin", bufs=3)
        g_pool = tc.alloc_tile_pool(name="gath", bufs=2)
        v_pool = tc.alloc_tile_pool(name="valp", bufs=2)
        vt_pool = tc.alloc_tile_pool(name="valT", bufs=1)
        o2_pool = tc.alloc_tile_pool(name="o2", bufs=2)
        psB = tc.alloc_tile_pool(name="psB", bufs=2, space="PSUM")
        psE = tc.alloc_tile_pool(name="psE", bufs=1, space="PSUM")


        def emit_rows(r_lo, r_hi):
            for r in range(r_lo, r_hi):
                rb = fin.tile([128, 2, 64], f32, tag="rb", name="rb")
                nc.vector.tensor_copy(out=rb[:, :, :],
                                      in_=xres[:, :, r * 64:(r + 1) * 64])
                for cc in range(2):
                    nc.vector.scalar_tensor_tensor(
                        out=rb[:, cc, 2:62],
                        in0=out3T[:, cc, r * 60:(r + 1) * 60],
                        scalar=c3[:, cc:cc + 1], in1=rb[:, cc, 2:62],
                        op0=OP.add, op1=OP.add)
                psf = psB.tile([64, 256], f32, space="PSUM", tag="fo", name="fo", bufs=1)
                for cc in range(2):
                    nc.tensor.transpose(out=psf[:, cc * 128:(cc + 1) * 128],
                                        in_=rb[:, cc, :], identity=idf[:])
                orow = fin.tile([64, 256], f32, tag="orow", name="orow")
                nc.scalar.activation(out=orow[:], in_=psf[:], func=AF.Relu,
                                     bias=zt[0:64, 0:1], scale=1.0)
                nc.sync.dma_start(out=outA_d[r * 64:(r + 1) * 64, :], in_=orow[:])

        ROWCUT = {0: (0, 8), 512: (8, 17), 1024: (17, 25), 1536: (25, 30)}
        idxap = idxp[:, :, :]
        for n0, nn in [(0, 512), (512, 512), (1024, 512), (1536, 384)]:
            valT = vt_pool.tile([128, 36, 512], bf16, tag="vt")
            vTap = valT[:, :, :]
            for gt in range(nn // 128):
                t = n0 // 128 + gt
                val = v_pool.tile([128, 18, 256], bf16, tag="val")
                for half9 in range(3):
                    G = g_pool.tile([128, 6, 1024], bf16, tag="G")
                    for j in range(6):
                        gk = half9 * 6 + j
                        iap = sap(idxap, 0, 128, t * idxap.ap[1][0] + gk, [[1, 1]])
                        nc.gpsimd.indirect_dma_start(
                            out=G[:, j, :], out_offset=None, in_=y2col,
                            in_offset=bass.IndirectOffsetOnAxis(ap=iap, axis=0))
                    for j in range(6):
                        gk = half9 * 6 + j
                        ab = At[:, t, gk * 4:gk * 4 + 4]
                        vv = val[:, gk, :]
                        nc.vector.tensor_scalar(
                            out=vv, in0=G[:, j, 768:1024], scalar1=ab[:, 3:4],
                            scalar2=None, op0=OP.mult)
                        for s_i, lo in ((2, 512), (1, 256), (0, 0)):
                            nc.vector.scalar_tensor_tensor(
                                out=vv, in0=G[:, j, lo:lo + 256],
                                scalar=ab[:, s_i:s_i + 1], in1=vv,
                                op0=OP.mult, op1=OP.add)
                for quad in range(9):
                    pst = psB.tile([128, 512], bf16, space="PSUM", tag="vtr", bufs=1)
                    for b4 in range(4):
                        sl = quad * 4 + b4
                        nc.tensor.transpose(
                            out=pst[:, b4 * 128:(b4 + 1) * 128],
                            in_=val[:, sl // 2, (sl % 2) * 128:(sl % 2) * 128 + 128],
                            identity=idb[:])
                    dst = sap(vTap, 0, 128, (quad * 4) * vTap.ap[1][0] + gt * 128,
                              [[vTap.ap[1][0], 4], [1, 128]])
                    nc.scalar.copy(out=dst, in_=pst[:])

            pse = psE.tile([128, 4, 512], f32, space="PSUM", tag="e")
            for fc in range(4):
                for sl in range(36):
                    nc.tensor.matmul(
                        out=pse[:, fc, :nn],
                        lhsT=wk[:, sl, fc * 128:(fc + 1) * 128],
                        rhs=valT[:, sl, :nn],
                        start=(sl == 0), stop=(sl == 35),
                    )
            out2T = o2_pool.tile([128, 4, 512], bf16, tag="o2t")
            for fc in range(4):
                nc.scalar.activation(
                    out=out2T[:, fc, :nn], in_=pse[:, fc, :nn], func=AF.Relu,
                    bias=c2[:, fc:fc + 1], scale=a2[:, fc:fc + 1])
            ps3 = psB.tile([128, 2, 512], f32, space="PSUM", tag="c3", bufs=1)
            for cc in range(2):
                for fc in range(4):
                    nc.tensor.matmul(
                        out=ps3[:, cc, :nn],
                        lhsT=w3[:, fc, cc * 128:(cc + 1) * 128],
                        rhs=out2T[:, fc, :nn],
                        start=(fc == 0), stop=(fc == 3),
                    )
            for cc in range(2):
                nc.scalar.activation(
                    out=out3T[:, cc, n0:n0 + nn], in_=ps3[:, cc, :nn],
                    func=AF.Copy, bias=0.0, scale=a3[:, cc:cc + 1])
            emit_rows(*ROWCUT[n0])

        for p in (psE, psB, o2_pool, vt_pool, v_pool, g_pool):
            p.release()

        # ===== Phase 4 tail: border rows =====
        psF = tc.alloc_tile_pool(name="psF", bufs=2, space="PSUM")
        for r in range(2):
            psf = psF.tile([64, 256], f32, space="PSUM", tag="fo")
            for cc in range(2):
                nc.tensor.transpose(out=psf[:, cc * 128:(cc + 1) * 128],
                                    in_=xbrd[:, cc, r * 64:(r + 1) * 64],
                                    identity=idf[:])
            orow = fin.tile([64, 256], f32, tag="orow")
            nc.scalar.activation(out=orow[:], in_=psf[:], func=AF.Relu,
                                 bias=zt[0:64, 0:1], scale=1.0)
            nc.sync.dma_start(out=outB_d[r * 64:(r + 1) * 64, :], in_=orow[:])
        psF.release()
        fin.release()
        cp.release()

    nc.finalize()
    return nc


def make_inputs(core, x, w1, s1, b1, m1, v1, w_off, b_off, w_d, s2, b2, m2, v2,
                w3, s3, b3, m3, v3):
    b, half = core // 2, core % 2
    h0 = half * 30
    bfl = ml_dtypes.bfloat16
    xb = np.ascontiguousarray(x[b].reshape(4096, 256).T)       # [256, 4096]
    xtb = np.ascontiguousarray(
        xb.reshape(2, 128, 4096).transpose(1, 0, 2).reshape(128, 2 * 4096)
    ).astype(bfl)
    xs = np.ascontiguousarray(x[b][h0:h0 + 34].reshape(34 * 64, 256).T)
    xts = np.ascontiguousarray(
        xs.reshape(2, 128, 2176).transpose(1, 0, 2).reshape(128, -1)).astype(bfl)
    rows = x[b][h0 + 2: h0 + 32].reshape(NROW * 64, 256).T      # [256, 1920]
    xres = np.ascontiguousarray(
        rows.reshape(2, 128, NROW * 64).transpose(1, 0, 2).reshape(128, -1))
    brows = [0, 1] if half == 0 else [62, 63]
    bd = x[b][brows].reshape(128, 256).T
    xbrd = np.ascontiguousarray(bd.reshape(2, 128, 128).transpose(1, 0, 2)
                                .reshape(128, 256))

    w1b = np.ascontiguousarray(
        w1[0, 0].reshape(2, 128, 512).transpose(1, 0, 2).reshape(128, -1)).astype(bfl)
    wo = w_off.reshape(9, 512, 54)
    woffh = np.zeros((128, 36, 54), np.float32)
    for tap in range(9):
        for cc in range(4):
            woffh[:, tap * 4 + cc, :] = wo[tap, cc * 128:(cc + 1) * 128, :]
    woffh = woffh.reshape(128, -1).astype(bfl)
    biah = b_off.reshape(54, 1).astype(np.float32)
    wkr = w_d.reshape(9, 512, 512)
    wkh = np.zeros((128, 36, 512), np.float32)
    for g in range(2):
        for k in range(9):
            gk = g * 9 + k
            for ch in range(2):
                wkh[:, gk * 2 + ch, :] = wkr[k, g * 256 + ch * 128:
                                             g * 256 + (ch + 1) * 128, :]
    wkh = wkh.reshape(128, -1).astype(bfl)
    w3h = np.ascontiguousarray(
        w3[0, 0].reshape(4, 128, 256).transpose(1, 0, 2).reshape(128, -1)).astype(bfl)

    def bn(s, bb, m, v, k):
        out = np.zeros((128, 4 * k), np.float32)
        for i, arr in enumerate([s, bb, m, v]):
            out[:, i * k:(i + 1) * k] = np.asarray(arr).reshape(k, 128).T
        return out

    bn1 = bn(s1, b1, m1, v1, 4)
    bn2 = bn(s2, b2, m2, v2, 4)
    bn3 = bn(s3, b3, m3, v3, 2)

    pybh = np.full((128, NT, 18), 1.0e6, np.float32)
    pxbh = np.full((128, NT, 18), 1.0e6, np.float32)
    gk = np.arange(18)
    kyl = ((gk % 9) // 3) * 2.0
    kxl = ((gk % 9) % 3) * 2.0
    pix = np.arange(NP)
    hh = h0 + pix // 60
    ww = pix % 60
    for t in range(NT):
        n = min(128, NP - t * 128)
        if n > 0:
            pybh[:n, t, :] = hh[t * 128:t * 128 + n, None] + kyl[None, :] + 64.0
            pxbh[:n, t, :] = ww[t * 128:t * 128 + n, None] + kxl[None, :] + 64.0
    gofh = np.zeros((128, 18), np.float32)
    gofh[:] = ((gk // 9) * 4096 - (64 * 64 + 64)).astype(np.float32)[None, :]

    return {
        "xtb": xtb, "xts": xts, "xres": xres.astype(np.float32),
        "xbrd": xbrd.astype(np.float32),
        "w1b": w1b, "woff": woffh, "bia": biah, "wk": wkh, "w3b": w3h,
        "bn1": bn1, "bn2": bn2, "bn3": bn3,
        "pyb": pybh.reshape(128, -1), "pxb": pxbh.reshape(128, -1), "gof": gofh,
        "rmk": np.concatenate([np.zeros(36, np.uint8),
                               np.ones(18, np.uint8)]).reshape(54, 1),
        "idf": np.eye(128, dtype=np.float32),
        "idb": np.eye(128, dtype=np.float32).astype(bfl),
    }


def kernel(**inputs):
    if "nc" not in _CACHE:
        _CACHE["nc"] = build_nc()
    nc = _CACHE["nc"]
    inputs = {k: np.asarray(v) for k, v in inputs.items()}
    in_maps = [make_inputs(core, **inputs) for core in range(8)]
    res = run_bass_kernel_spmd(nc, in_maps, list(range(8)))
    out = np.zeros((4, 64, 64, 256), np.float32)
    for core in range(8):
        b, half = core // 2, core % 2
        r = res.results[core]
        oa = r["outA"].reshape(NROW, 64, 256)
        ob = r["outB"].reshape(2, 64, 256)
        out[b, half * 30 + 2: half * 30 + 32] = oa
        if half == 0:
            out[b, 0:2] = ob
        else:
            out[b, 62:64] = ob
    return out
